# revision 20
# baseline (speedup 1.0000x reference)
"""CrossAttentionFusion Bass kernel v2 — feature-major end-to-end.

Reference (T=4096, B=64, D=64):
    q = eeg @ Wq.T + bq ; k = fnirs @ Wk.T + bk ; v = fnirs @ Wv.T + bv
    s = sum(q*k, -1) * D**-0.5 ; a = softmax(s, axis=0) ; out = eeg + a*v

Algebra: s = SCALE*(x^T G y + u.y + w.x), G = Wq^T Wk, u = Wk^T bq,
w = Wq^T bk (constant bq.bk dropped - softmax invariant).

Per core: 8 batches as 4 pairs; feature-major pair-packed tiles
[128 = 64f x 2 batches, tokens]. Token-chunks of 1024 (4 per pair).

Phase A (per pair p, block tb):
  z = BD(G)^T @ x        (PE -> PSUM)
  zs = z + u2            (ACT Identity, bias=u2, bf16 out)
  m = zs * y             (DVE tt 2x bf16)
  s[8rows@32*tb] += red8_p^T @ m + wsel_p^T @ x   (PE, rows 2p,2p+1)
  after p==3: E = exp(s) (ACT on 8-row slices, accum Z-partials)
Z: 4 selector matmuls -> Zrep per pair layout; DVE reciprocal -> rz2_p.
Phase B (per p, tb):
  Eb = onesel_p^T @ E-rows   (PE broadcast of unnormalized exp)
  ya = Eb * y                (DVE/ACT-route/GPS-route)
  v' = BD(Wv^T)^T @ ya + bvsel_p^T @ E-rows  (PE)
  o = (v' * rz2_p) + x       (DVE stt; rz per-partition batch-half)
  DMA out (feature-major bf16; host unpacks).
"""

import sys

sys.path.insert(0, "/opt/trn_rl_repo")

import ml_dtypes
import numpy as np

import concourse.bass as bass
import concourse.tile as tile
from concourse import bacc, mybir

T, B, D = 4096, 64, 64
N_CORES = 8
BC = B // N_CORES  # 8 batches per core
NP = BC // 2  # 4 pairs
NTB = 4  # token blocks of 1024
CH = T // NTB  # 1024
SCALE = float(D) ** -0.5
F32 = mybir.dt.float32
BF16 = mybir.dt.bfloat16
NPBF16 = ml_dtypes.bfloat16
AF = mybir.ActivationFunctionType
ALU = mybir.AluOpType

# routing: which (p, tb) chunk indices (0..15) take the ACT-evac path
YA_ACT = {1, 3, 5, 7, 9, 11, 13, 15}
O_ACT = {0, 2, 4, 6, 8, 10, 12, 14}

_CACHE = {}


def _build_nc():
    nc = bacc.Bacc(
        "TRN2", target_bir_lowering=False, debug=False, num_devices=N_CORES
    )

    xp_d = nc.dram_tensor("XP", [NP, 128, NTB * CH], BF16, kind="ExternalInput").ap()
    yp_d = nc.dram_tensor("YP", [NP, 128, NTB * CH], BF16, kind="ExternalInput").ap()
    bdg_d = nc.dram_tensor("BDG", [128, 128], BF16, kind="ExternalInput").ap()
    u2_d = nc.dram_tensor("U2", [128, 1], F32, kind="ExternalInput").ap()
    redw_d = nc.dram_tensor("REDW", [128, NP * 16], BF16, kind="ExternalInput").ap()
    onesel_d = nc.dram_tensor("ONESEL", [128, NP * 128], BF16, kind="ExternalInput").ap()
    zsel_d = nc.dram_tensor("ZSEL", [128, NP * 128], F32, kind="ExternalInput").ap()
    bdwv_d = nc.dram_tensor("BDWV", [128, 128], BF16, kind="ExternalInput").ap()
    out_d = nc.dram_tensor("OUT", [NP, 128, NTB * CH], BF16, kind="ExternalOutput").ap()

    with tile.TileContext(nc) as tc:
        with (
            tc.tile_pool(name="consts", bufs=1) as consts,
            tc.tile_pool(name="store", bufs=1) as store,
            tc.tile_pool(name="zs", bufs=3) as zsp,
            tc.tile_pool(name="m", bufs=3) as mp,
            tc.tile_pool(name="eb", bufs=4) as ebp,
            tc.tile_pool(name="ya", bufs=16) as yap,
            tc.tile_pool(name="vs", bufs=3) as vsp,
            tc.tile_pool(name="o", bufs=3) as op_,
            tc.tile_pool(name="small", bufs=1) as smalls,
            # PSUM: pza 2x[128,1024]f32 = 4 banks (z in A, Eb in B)
            #       psv 2x[128,1024]f32 = 4 banks (s + Zrep in A, v in B)
            tc.tile_pool(name="pza", bufs=2, space="PSUM") as pza,
            tc.tile_pool(name="psv", bufs=2, space="PSUM") as psv,
        ):
            bdg_s = consts.tile([128, 128], BF16)
            nc.scalar.dma_start(bdg_s[:], bdg_d[:])
            u2_s = consts.tile([128, 1], F32)
            nc.scalar.dma_start(u2_s[:], u2_d[:])
            redw_s = consts.tile([128, NP * 16], BF16)
            nc.scalar.dma_start(redw_s[:], redw_d[:])
            onesel_s = consts.tile([128, NP * 128], BF16)
            nc.scalar.dma_start(onesel_s[:], onesel_d[:])
            zsel_s = consts.tile([128, NP * 128], F32)
            nc.scalar.dma_start(zsel_s[:], zsel_d[:])
            bdwv_s = consts.tile([128, 128], BF16)
            nc.scalar.dma_start(bdwv_s[:], bdwv_d[:])

            xstore = store.tile([128, NP * NTB * CH], BF16)  # 32 KB/part
            ystore = store.tile([128, NP * NTB * CH], BF16)  # 32 KB/part
            ostore = store.tile([128, NP * NTB * CH], BF16)  # 32 KB/part
            # E rows: tile tb//2, partition base 64*(tb%2), rows +2p+bi
            e_s = [store.tile([128, CH], BF16, name=f"e{h}") for h in range(2)]
            zp_s = smalls.tile([128, NTB], F32)  # Z partials per tb col
            rz2 = smalls.tile([128, NP], F32)  # 1/Z per pair, partition-half

            def csl(p, tb):
                base = (p * NTB + tb) * CH
                return slice(base, base + CH)

            nc.vector.memset(zp_s[:], 0.0)

            def mm512(out, lhsT, rhs, start, stop, **kw):
                # matmul out free dim must fit one PSUM bank (512 f32)
                n = out.shape[-1]
                for c0 in range(0, n, 512):
                    c1 = min(c0 + 512, n)
                    nc.tensor.matmul(
                        out[:, c0:c1], lhsT, rhs[:, c0:c1],
                        start=start, stop=stop, **kw,
                    )

            # s psum: tile tb//2, rows (2p+bi)@64*(tb%2); cols = token in block
            s_ps = [psv.tile([128, CH], F32, tag="sv", name=f"s{h}") for h in range(2)]

            # input DMAs: x on sync ring, y on gpsimd ring; first pair
            # split per-chunk so compute can start on the first block
            for tb in range(NTB):
                cs = slice(tb * CH, (tb + 1) * CH)
                nc.sync.dma_start(xstore[:, cs], xp_d[0, :, cs])
                nc.gpsimd.dma_start(ystore[:, cs], yp_d[0, :, cs])
            for p in range(1, NP):
                psl = slice(p * NTB * CH, (p + 1) * NTB * CH)
                nc.sync.dma_start(xstore[:, psl], xp_d[p])
                nc.gpsimd.dma_start(ystore[:, psl], yp_d[p])

            # ---------------- Phase A (lag-2 software pipeline on PE) -----
            LAG = 2
            m_tiles = {}

            def a_front(i):
                p, tb = divmod(i, NTB)
                sl = csl(p, tb)
                zq = pza.tile([128, CH], F32, tag="za", name="zq")
                mm512(zq, bdg_s[:], xstore[:, sl], start=True, stop=True)
                zs = zsp.tile([128, CH], BF16, tag="zs")
                nc.scalar.activation(
                    zs[:], zq[:], AF.Identity, bias=u2_s[:, 0:1], scale=1.0
                )
                m = mp.tile([128, CH], BF16, tag="m")
                nc.vector.tensor_tensor(m[:], zs[:], ystore[:, sl], op=ALU.mult)
                m_tiles[i] = m

            def a_back(i):
                p, tb = divmod(i, NTB)
                sl = csl(p, tb)
                m = m_tiles.pop(i)
                sh, prow = s_ps[tb // 2], slice(64 * (tb % 2), 64 * (tb % 2) + 8)
                mm512(
                    sh[prow, :], redw_s[:, p * 16 : p * 16 + 8], m[:],
                    start=(p == 0), stop=False, skip_group_check=True,
                )
                mm512(
                    sh[prow, :], redw_s[:, p * 16 + 8 : p * 16 + 16],
                    xstore[:, sl],
                    start=False, stop=(p == 3), skip_group_check=True,
                )
                if p == 3:
                    nc.scalar.activation(
                        e_s[tb // 2][prow, :], sh[prow, :], AF.Exp,
                        bias=0.0, scale=1.0,
                        accum_out=zp_s[prow, tb : tb + 1],
                    )

            for i in range(NP * NTB + LAG):
                if i < NP * NTB:
                    a_front(i)
                if i >= LAG:
                    a_back(i - LAG)

            # Z -> rz2 block, emitted inside phase B (see emit_z below) so
            # eb matmuls can start on PE before the Z reduction completes
            def emit_z():
                zp1 = smalls.tile([128, 1], F32)
                nc.vector.tensor_reduce(
                    zp1[:], zp_s[:], axis=mybir.AxisListType.X, op=ALU.add
                )
                zrep = pza.tile([128, CH], F32, tag="za", name="zrep")
                for p in range(NP):
                    nc.tensor.matmul(
                        zrep[:, p : p + 1],
                        zsel_s[:, p * 128 : (p + 1) * 128],
                        zp1[:, 0:1],
                        start=True, stop=True, skip_group_check=True,
                    )
                nc.vector.reciprocal(rz2[:, 0:NP], zrep[:, 0:NP])

            # ---------------- Phase B ------------------------------------
            # b_front(tb, p): Eb broadcast via SBUF->SBUF DMA (two E rows
            # replicated across the 64-partition halves), then ya = Eb * y
            # (DVE 2x). Emitted eagerly right after exp(tb) so it overlaps
            # the tail of phase A. b_back needs rz (full-T softmax sum).
            ya_tiles = {}

            def b_front(tb, p):
                ci = p * NTB + tb
                sl = csl(p, tb)
                et = e_s[tb // 2]
                prow = slice(64 * (tb % 2), 64 * (tb % 2) + 8)
                eb = pza.tile([128, CH], F32, tag="za", name="eb")
                mm512(
                    eb, onesel_s[prow, p * 128 : (p + 1) * 128],
                    et[prow, :], start=True, stop=True,
                )
                ya = yap.tile([128, CH], BF16, tag="ya")
                if ci in YA_ACT:
                    ebs = ebp.tile([128, CH], BF16, tag="ebs")
                    nc.scalar.activation(
                        ebs[:], eb[:], AF.Copy, bias=0.0, scale=1.0
                    )
                    nc.vector.tensor_tensor(
                        ya[:], ebs[:], ystore[:, sl], op=ALU.mult
                    )
                else:
                    nc.vector.tensor_tensor(
                        ya[:], eb[:], ystore[:, sl], op=ALU.mult
                    )
                ya_tiles[(tb, p)] = ya

            def b_back(tb, p):
                ci = p * NTB + tb
                sl = csl(p, tb)
                ya = ya_tiles.pop((tb, p))
                vps = psv.tile([128, CH], F32, tag="sv", name="vps")
                mm512(vps, bdwv_s[:], ya[:], start=True, stop=True)
                if ci in O_ACT:
                    vsn = vsp.tile([128, CH], BF16, tag="vsn")
                    nc.scalar.activation(
                        vsn[:], vps[:], AF.Copy, bias=0.0,
                        scale=rz2[:, p : p + 1],
                    )
                    nc.vector.tensor_tensor(
                        ostore[:, sl], vsn[:], xstore[:, sl], op=ALU.add
                    )
                else:
                    nc.vector.scalar_tensor_tensor(
                        ostore[:, sl], vps[:], rz2[:, p : p + 1],
                        xstore[:, sl],
                        op0=ALU.mult, op1=ALU.add,
                    )
                nc.gpsimd.dma_start(out_d[p, :, tb * CH : (tb + 1) * CH],
                                    ostore[:, sl])

            for tb in range(NTB):
                for p in range(NP):
                    b_front(tb, p)
                if tb == 2:
                    emit_z()
            for tb in range(NTB):
                for p in range(NP):
                    b_back(tb, p)

    nc.compile()
    return nc


def _get_nc():
    if "nc" not in _CACHE:
        _CACHE["nc"] = _build_nc()
    return _CACHE["nc"]


def _host_constants(Wq, bq, Wk, bk, Wv, bv):
    Wq64, Wk64, Wv64 = (np.asarray(a, np.float64) for a in (Wq, Wk, Wv))
    bq64, bk64, bv64 = (np.asarray(a, np.float64) for a in (bq, bk, bv))
    # bv fold: y'' = y + cv with cv = Wv^-1 bv, so v = Wv y'' exactly;
    # scores compensate via w' = w + G cv (u.cv constant drops in softmax)
    cv = np.linalg.solve(Wv64, bv64)
    G = SCALE * (Wq64.T @ Wk64)
    u = SCALE * (Wk64.T @ bq64)
    w = SCALE * (Wq64.T @ bk64) - G @ cv

    BDG = np.zeros((128, 128), np.float64)
    BDG[0:64, 0:64] = G
    BDG[64:128, 64:128] = G
    U2 = np.concatenate([u, u]).reshape(128, 1).astype(np.float32)
    # REDW: per pair p, cols [16p:16p+8] = m-reduce selector (col r = 2p+bi
    # -> ones at partitions bi*64..), cols [16p+8:16p+16] = w-vector selector
    REDW = np.zeros((128, NP * 16), np.float64)
    for p in range(NP):
        for bi in range(2):
            r = 2 * p + bi
            REDW[bi * 64 : (bi + 1) * 64, 16 * p + r] = 1.0
            REDW[bi * 64 : (bi + 1) * 64, 16 * p + 8 + r] = w
    # ONESEL: [128, 128] per pair, replicated at 2 64-row groups:
    # lhsT[64g + r, bi*64 + f] = 1 iff r == 2p+bi
    ONESEL = np.zeros((128, NP * 128), np.float64)
    for p in range(NP):
        for g in range(2):
            for bi in range(2):
                r = 64 * g + 2 * p + bi
                fs = slice(p * 128 + bi * 64, p * 128 + (bi + 1) * 64)
                ONESEL[r, fs] = 1.0
    # ZSEL: per pair p: zsel_p[64g + 2p + bi, bi*64 + f] = 1 (both groups;
    # zp rows outside the written group are zeroed by memset)
    ZSEL = np.zeros((128, NP * 128), np.float64)
    for p in range(NP):
        for g in range(2):
            for bi in range(2):
                ZSEL[64 * g + 2 * p + bi, p * 128 + bi * 64 : p * 128 + (bi + 1) * 64] = 1.0
    BDWV = np.zeros((128, 128), np.float64)
    BDWV[0:64, 0:64] = Wv64.T
    BDWV[64:128, 64:128] = Wv64.T
    return (
        BDG.astype(NPBF16), U2, REDW.astype(NPBF16),
        ONESEL.astype(NPBF16),
        ZSEL.astype(np.float32), BDWV.astype(NPBF16), cv,
    )


def _pack_inputs(eeg, fnirs, cv):
    # [core, p, tb, 128 = bi*64 + d, j]; token t = tb*1024 + j
    e = np.asarray(eeg, np.float32).reshape(NTB, CH, N_CORES, NP, 2, D)
    f = (np.asarray(fnirs, np.float64) + cv).astype(np.float32)
    f = f.reshape(NTB, CH, N_CORES, NP, 2, D)
    XP = np.ascontiguousarray(e.transpose(2, 3, 0, 4, 5, 1)).astype(NPBF16)
    YP = np.ascontiguousarray(f.transpose(2, 3, 0, 4, 5, 1)).astype(NPBF16)
    # [core, p, tb, 128, j] -> [core, p, 128, tb*CH]
    XP = XP.reshape(N_CORES, NP, NTB, 128, CH).transpose(0, 1, 3, 2, 4)
    YP = YP.reshape(N_CORES, NP, NTB, 128, CH).transpose(0, 1, 3, 2, 4)
    return (
        np.ascontiguousarray(XP).reshape(N_CORES, NP, 128, NTB * CH),
        np.ascontiguousarray(YP).reshape(N_CORES, NP, 128, NTB * CH),
    )


def _unpack_output(outs):
    o = np.stack(outs).astype(np.float32)  # [core, p, 128, tb*CH]
    o = o.reshape(N_CORES, NP, 2, D, NTB, CH)
    o = o.transpose(4, 5, 0, 1, 2, 3)  # [tb, j, core, p, bi, d]
    return np.ascontiguousarray(o.reshape(T, B, D))


def _prepare(eeg, fnirs, Wq, bq, Wk, bk, Wv, bv):
    BDG, U2, REDW, ONESEL, ZSEL, BDWV, cv = _host_constants(
        Wq, bq, Wk, bk, Wv, bv
    )
    XP, YP = _pack_inputs(eeg, fnirs, cv)
    return [
        {
            "XP": XP[c], "YP": YP[c], "BDG": BDG, "U2": U2, "REDW": REDW,
            "ONESEL": ONESEL, "ZSEL": ZSEL, "BDWV": BDWV,
        }
        for c in range(N_CORES)
    ]


def _run(eeg, fnirs, Wq, bq, Wk, bk, Wv, bv, **spmd_kwargs):
    from concourse.bass_utils import run_bass_kernel_spmd

    nc = _get_nc()
    in_maps = _prepare(eeg, fnirs, Wq, bq, Wk, bk, Wv, bv)
    res = run_bass_kernel_spmd(nc, in_maps, list(range(N_CORES)), **spmd_kwargs)
    return _unpack_output([res.results[c]["OUT"] for c in range(N_CORES)]), res


def kernel(eeg, fnirs, Wq, bq, Wk, bk, Wv, bv):
    return _run(eeg, fnirs, Wq, bq, Wk, bk, Wv, bv)[0]


# revision 21
# speedup vs baseline: 1.0094x; 1.0094x over previous
"""CrossAttentionFusion Bass kernel v2 — feature-major end-to-end.

Reference (T=4096, B=64, D=64):
    q = eeg @ Wq.T + bq ; k = fnirs @ Wk.T + bk ; v = fnirs @ Wv.T + bv
    s = sum(q*k, -1) * D**-0.5 ; a = softmax(s, axis=0) ; out = eeg + a*v

Algebra: s = SCALE*(x^T G y + u.y + w.x), G = Wq^T Wk, u = Wk^T bq,
w = Wq^T bk (constant bq.bk dropped - softmax invariant).

Per core: 8 batches as 4 pairs; feature-major pair-packed tiles
[128 = 64f x 2 batches, tokens]. Token-chunks of 1024 (4 per pair).

Phase A (per pair p, block tb):
  z = BD(G)^T @ x        (PE -> PSUM)
  zs = z + u2            (ACT Identity, bias=u2, bf16 out)
  m = zs * y             (DVE tt 2x bf16)
  s[8rows@32*tb] += red8_p^T @ m + wsel_p^T @ x   (PE, rows 2p,2p+1)
  after p==3: E = exp(s) (ACT on 8-row slices, accum Z-partials)
Z: 4 selector matmuls -> Zrep per pair layout; DVE reciprocal -> rz2_p.
Phase B (per p, tb):
  Eb = onesel_p^T @ E-rows   (PE broadcast of unnormalized exp)
  ya = Eb * y                (DVE/ACT-route/GPS-route)
  v' = BD(Wv^T)^T @ ya + bvsel_p^T @ E-rows  (PE)
  o = (v' * rz2_p) + x       (DVE stt; rz per-partition batch-half)
  DMA out (feature-major bf16; host unpacks).
"""

import sys

sys.path.insert(0, "/opt/trn_rl_repo")

import ml_dtypes
import numpy as np

import concourse.bass as bass
import concourse.tile as tile
from concourse import bacc, mybir

T, B, D = 4096, 64, 64
N_CORES = 8
BC = B // N_CORES  # 8 batches per core
NP = BC // 2  # 4 pairs
NTB = 4  # token blocks of 1024
CH = T // NTB  # 1024
SCALE = float(D) ** -0.5
F32 = mybir.dt.float32
BF16 = mybir.dt.bfloat16
NPBF16 = ml_dtypes.bfloat16
AF = mybir.ActivationFunctionType
ALU = mybir.AluOpType

# routing: which (p, tb) chunk indices (0..15) take the ACT-evac path
YA_ACT = {1, 3, 5, 7, 9, 11, 13, 15}
O_ACT = {0, 2, 4, 6, 8, 10, 12, 14}

_CACHE = {}


def _build_nc():
    nc = bacc.Bacc(
        "TRN2", target_bir_lowering=False, debug=False, num_devices=N_CORES
    )

    xp_d = nc.dram_tensor("XP", [NP, 128, NTB * CH], BF16, kind="ExternalInput").ap()
    yp_d = nc.dram_tensor("YP", [NP, 128, NTB * CH], BF16, kind="ExternalInput").ap()
    bdg_d = nc.dram_tensor("BDG", [128, 128], BF16, kind="ExternalInput").ap()
    u2_d = nc.dram_tensor("U2", [128, 1], F32, kind="ExternalInput").ap()
    redw_d = nc.dram_tensor("REDW", [128, NP * 16], BF16, kind="ExternalInput").ap()
    onesel_d = nc.dram_tensor("ONESEL", [128, NP * 128], BF16, kind="ExternalInput").ap()
    zsel_d = nc.dram_tensor("ZSEL", [128, NP * 128], F32, kind="ExternalInput").ap()
    bdwv_d = nc.dram_tensor("BDWV", [128, 128], BF16, kind="ExternalInput").ap()
    out_d = nc.dram_tensor("OUT", [NP, 128, NTB * CH], BF16, kind="ExternalOutput").ap()

    with tile.TileContext(nc) as tc:
        with (
            tc.tile_pool(name="consts", bufs=1) as consts,
            tc.tile_pool(name="store", bufs=1) as store,
            tc.tile_pool(name="zs", bufs=3) as zsp,
            tc.tile_pool(name="m", bufs=3) as mp,
            tc.tile_pool(name="eb", bufs=4) as ebp,
            tc.tile_pool(name="ya", bufs=16) as yap,
            tc.tile_pool(name="vs", bufs=3) as vsp,
            tc.tile_pool(name="o", bufs=3) as op_,
            tc.tile_pool(name="small", bufs=1) as smalls,
            # PSUM: pza 2x[128,1024]f32 = 4 banks (z in A, Eb in B)
            #       psv 2x[128,1024]f32 = 4 banks (s + Zrep in A, v in B)
            tc.tile_pool(name="pza", bufs=2, space="PSUM") as pza,
            tc.tile_pool(name="psv", bufs=2, space="PSUM") as psv,
        ):
            bdg_s = consts.tile([128, 128], BF16)
            nc.scalar.dma_start(bdg_s[:], bdg_d[:])
            u2_s = consts.tile([128, 1], F32)
            nc.scalar.dma_start(u2_s[:], u2_d[:])
            redw_s = consts.tile([128, NP * 16], BF16)
            nc.scalar.dma_start(redw_s[:], redw_d[:])
            onesel_s = consts.tile([128, NP * 128], BF16)
            nc.scalar.dma_start(onesel_s[:], onesel_d[:])
            zsel_s = consts.tile([128, NP * 128], F32)
            nc.scalar.dma_start(zsel_s[:], zsel_d[:])
            bdwv_s = consts.tile([128, 128], BF16)
            nc.scalar.dma_start(bdwv_s[:], bdwv_d[:])

            xstore = store.tile([128, NP * NTB * CH], BF16)  # 32 KB/part
            ystore = store.tile([128, NP * NTB * CH], BF16)  # 32 KB/part
            ostore = store.tile([128, NP * NTB * CH], BF16)  # 32 KB/part
            # E rows: tile tb//2, partition base 64*(tb%2), rows +2p+bi
            e_s = [store.tile([128, CH], BF16, name=f"e{h}") for h in range(2)]
            zp_s = smalls.tile([128, NTB], F32)  # Z partials per tb col
            rz2 = smalls.tile([128, NP], F32)  # 1/Z per pair, partition-half

            def csl(p, tb):
                base = (p * NTB + tb) * CH
                return slice(base, base + CH)

            nc.vector.memset(zp_s[:], 0.0)

            def mm512(out, lhsT, rhs, start, stop, **kw):
                # matmul out free dim must fit one PSUM bank (512 f32)
                n = out.shape[-1]
                for c0 in range(0, n, 512):
                    c1 = min(c0 + 512, n)
                    nc.tensor.matmul(
                        out[:, c0:c1], lhsT, rhs[:, c0:c1],
                        start=start, stop=stop, **kw,
                    )

            # s psum: tile tb//2, rows (2p+bi)@64*(tb%2); cols = token in block
            s_ps = [psv.tile([128, CH], F32, tag="sv", name=f"s{h}") for h in range(2)]

            # input DMAs: x on sync ring, y on gpsimd ring. Keep
            # descriptors >= 4 KB/partition; first pair in halves so the
            # first block lands early
            for h in range(2):
                cs = slice(h * 2 * CH, (h + 1) * 2 * CH)
                nc.sync.dma_start(xstore[:, cs], xp_d[0, :, cs])
                nc.gpsimd.dma_start(ystore[:, cs], yp_d[0, :, cs])
            for p in range(1, NP):
                psl = slice(p * NTB * CH, (p + 1) * NTB * CH)
                nc.sync.dma_start(xstore[:, psl], xp_d[p])
                nc.gpsimd.dma_start(ystore[:, psl], yp_d[p])

            # ---------------- Phase A (lag-2 software pipeline on PE) -----
            LAG = 2
            m_tiles = {}

            def a_front(i):
                p, tb = divmod(i, NTB)
                sl = csl(p, tb)
                zq = pza.tile([128, CH], F32, tag="za", name="zq")
                mm512(zq, bdg_s[:], xstore[:, sl], start=True, stop=True)
                zs = zsp.tile([128, CH], BF16, tag="zs")
                nc.scalar.activation(
                    zs[:], zq[:], AF.Identity, bias=u2_s[:, 0:1], scale=1.0
                )
                m = mp.tile([128, CH], BF16, tag="m")
                nc.vector.tensor_tensor(m[:], zs[:], ystore[:, sl], op=ALU.mult)
                m_tiles[i] = m

            def a_back(i):
                p, tb = divmod(i, NTB)
                sl = csl(p, tb)
                m = m_tiles.pop(i)
                sh, prow = s_ps[tb // 2], slice(64 * (tb % 2), 64 * (tb % 2) + 8)
                mm512(
                    sh[prow, :], redw_s[:, p * 16 : p * 16 + 8], m[:],
                    start=(p == 0), stop=False, skip_group_check=True,
                )
                mm512(
                    sh[prow, :], redw_s[:, p * 16 + 8 : p * 16 + 16],
                    xstore[:, sl],
                    start=False, stop=(p == 3), skip_group_check=True,
                )
                if p == 3:
                    nc.scalar.activation(
                        e_s[tb // 2][prow, :], sh[prow, :], AF.Exp,
                        bias=0.0, scale=1.0,
                        accum_out=zp_s[prow, tb : tb + 1],
                    )

            for i in range(NP * NTB + LAG):
                if i < NP * NTB:
                    a_front(i)
                if i >= LAG:
                    a_back(i - LAG)

            # Z -> rz2 block, emitted inside phase B (see emit_z below) so
            # eb matmuls can start on PE before the Z reduction completes
            def emit_z():
                zp1 = smalls.tile([128, 1], F32)
                nc.vector.tensor_reduce(
                    zp1[:], zp_s[:], axis=mybir.AxisListType.X, op=ALU.add
                )
                zrep = pza.tile([128, CH], F32, tag="za", name="zrep")
                for p in range(NP):
                    nc.tensor.matmul(
                        zrep[:, p : p + 1],
                        zsel_s[:, p * 128 : (p + 1) * 128],
                        zp1[:, 0:1],
                        start=True, stop=True, skip_group_check=True,
                    )
                nc.vector.reciprocal(rz2[:, 0:NP], zrep[:, 0:NP])

            # ---------------- Phase B ------------------------------------
            # b_front(tb, p): Eb broadcast via SBUF->SBUF DMA (two E rows
            # replicated across the 64-partition halves), then ya = Eb * y
            # (DVE 2x). Emitted eagerly right after exp(tb) so it overlaps
            # the tail of phase A. b_back needs rz (full-T softmax sum).
            ya_tiles = {}

            def b_front(tb, p):
                ci = p * NTB + tb
                sl = csl(p, tb)
                et = e_s[tb // 2]
                prow = slice(64 * (tb % 2), 64 * (tb % 2) + 8)
                eb = pza.tile([128, CH], F32, tag="za", name="eb")
                mm512(
                    eb, onesel_s[prow, p * 128 : (p + 1) * 128],
                    et[prow, :], start=True, stop=True,
                )
                ya = yap.tile([128, CH], BF16, tag="ya")
                if ci in YA_ACT:
                    ebs = ebp.tile([128, CH], BF16, tag="ebs")
                    nc.scalar.activation(
                        ebs[:], eb[:], AF.Copy, bias=0.0, scale=1.0
                    )
                    nc.vector.tensor_tensor(
                        ya[:], ebs[:], ystore[:, sl], op=ALU.mult
                    )
                else:
                    nc.vector.tensor_tensor(
                        ya[:], eb[:], ystore[:, sl], op=ALU.mult
                    )
                ya_tiles[(tb, p)] = ya

            def b_back(tb, p):
                ci = p * NTB + tb
                sl = csl(p, tb)
                ya = ya_tiles.pop((tb, p))
                vps = psv.tile([128, CH], F32, tag="sv", name="vps")
                mm512(vps, bdwv_s[:], ya[:], start=True, stop=True)
                if ci in O_ACT:
                    vsn = vsp.tile([128, CH], BF16, tag="vsn")
                    nc.scalar.activation(
                        vsn[:], vps[:], AF.Copy, bias=0.0,
                        scale=rz2[:, p : p + 1],
                    )
                    nc.vector.tensor_tensor(
                        ostore[:, sl], vsn[:], xstore[:, sl], op=ALU.add
                    )
                else:
                    nc.vector.scalar_tensor_tensor(
                        ostore[:, sl], vps[:], rz2[:, p : p + 1],
                        xstore[:, sl],
                        op0=ALU.mult, op1=ALU.add,
                    )
                if tb == NTB - 1:
                    psl = slice(p * NTB * CH, (p + 1) * NTB * CH)
                    nc.gpsimd.dma_start(out_d[p], ostore[:, psl])

            for tb in range(NTB):
                for p in range(NP):
                    b_front(tb, p)
                if tb == 2:
                    emit_z()
            for tb in range(NTB):
                for p in range(NP):
                    b_back(tb, p)

    nc.compile()
    return nc


def _get_nc():
    if "nc" not in _CACHE:
        _CACHE["nc"] = _build_nc()
    return _CACHE["nc"]


def _host_constants(Wq, bq, Wk, bk, Wv, bv):
    Wq64, Wk64, Wv64 = (np.asarray(a, np.float64) for a in (Wq, Wk, Wv))
    bq64, bk64, bv64 = (np.asarray(a, np.float64) for a in (bq, bk, bv))
    # bv fold: y'' = y + cv with cv = Wv^-1 bv, so v = Wv y'' exactly;
    # scores compensate via w' = w + G cv (u.cv constant drops in softmax)
    cv = np.linalg.solve(Wv64, bv64)
    G = SCALE * (Wq64.T @ Wk64)
    u = SCALE * (Wk64.T @ bq64)
    w = SCALE * (Wq64.T @ bk64) - G @ cv

    BDG = np.zeros((128, 128), np.float64)
    BDG[0:64, 0:64] = G
    BDG[64:128, 64:128] = G
    U2 = np.concatenate([u, u]).reshape(128, 1).astype(np.float32)
    # REDW: per pair p, cols [16p:16p+8] = m-reduce selector (col r = 2p+bi
    # -> ones at partitions bi*64..), cols [16p+8:16p+16] = w-vector selector
    REDW = np.zeros((128, NP * 16), np.float64)
    for p in range(NP):
        for bi in range(2):
            r = 2 * p + bi
            REDW[bi * 64 : (bi + 1) * 64, 16 * p + r] = 1.0
            REDW[bi * 64 : (bi + 1) * 64, 16 * p + 8 + r] = w
    # ONESEL: [128, 128] per pair, replicated at 2 64-row groups:
    # lhsT[64g + r, bi*64 + f] = 1 iff r == 2p+bi
    ONESEL = np.zeros((128, NP * 128), np.float64)
    for p in range(NP):
        for g in range(2):
            for bi in range(2):
                r = 64 * g + 2 * p + bi
                fs = slice(p * 128 + bi * 64, p * 128 + (bi + 1) * 64)
                ONESEL[r, fs] = 1.0
    # ZSEL: per pair p: zsel_p[64g + 2p + bi, bi*64 + f] = 1 (both groups;
    # zp rows outside the written group are zeroed by memset)
    ZSEL = np.zeros((128, NP * 128), np.float64)
    for p in range(NP):
        for g in range(2):
            for bi in range(2):
                ZSEL[64 * g + 2 * p + bi, p * 128 + bi * 64 : p * 128 + (bi + 1) * 64] = 1.0
    BDWV = np.zeros((128, 128), np.float64)
    BDWV[0:64, 0:64] = Wv64.T
    BDWV[64:128, 64:128] = Wv64.T
    return (
        BDG.astype(NPBF16), U2, REDW.astype(NPBF16),
        ONESEL.astype(NPBF16),
        ZSEL.astype(np.float32), BDWV.astype(NPBF16), cv,
    )


def _pack_inputs(eeg, fnirs, cv):
    # [core, p, tb, 128 = bi*64 + d, j]; token t = tb*1024 + j
    e = np.asarray(eeg, np.float32).reshape(NTB, CH, N_CORES, NP, 2, D)
    f = (np.asarray(fnirs, np.float64) + cv).astype(np.float32)
    f = f.reshape(NTB, CH, N_CORES, NP, 2, D)
    XP = np.ascontiguousarray(e.transpose(2, 3, 0, 4, 5, 1)).astype(NPBF16)
    YP = np.ascontiguousarray(f.transpose(2, 3, 0, 4, 5, 1)).astype(NPBF16)
    # [core, p, tb, 128, j] -> [core, p, 128, tb*CH]
    XP = XP.reshape(N_CORES, NP, NTB, 128, CH).transpose(0, 1, 3, 2, 4)
    YP = YP.reshape(N_CORES, NP, NTB, 128, CH).transpose(0, 1, 3, 2, 4)
    return (
        np.ascontiguousarray(XP).reshape(N_CORES, NP, 128, NTB * CH),
        np.ascontiguousarray(YP).reshape(N_CORES, NP, 128, NTB * CH),
    )


def _unpack_output(outs):
    o = np.stack(outs).astype(np.float32)  # [core, p, 128, tb*CH]
    o = o.reshape(N_CORES, NP, 2, D, NTB, CH)
    o = o.transpose(4, 5, 0, 1, 2, 3)  # [tb, j, core, p, bi, d]
    return np.ascontiguousarray(o.reshape(T, B, D))


def _prepare(eeg, fnirs, Wq, bq, Wk, bk, Wv, bv):
    BDG, U2, REDW, ONESEL, ZSEL, BDWV, cv = _host_constants(
        Wq, bq, Wk, bk, Wv, bv
    )
    XP, YP = _pack_inputs(eeg, fnirs, cv)
    return [
        {
            "XP": XP[c], "YP": YP[c], "BDG": BDG, "U2": U2, "REDW": REDW,
            "ONESEL": ONESEL, "ZSEL": ZSEL, "BDWV": BDWV,
        }
        for c in range(N_CORES)
    ]


def _run(eeg, fnirs, Wq, bq, Wk, bk, Wv, bv, **spmd_kwargs):
    from concourse.bass_utils import run_bass_kernel_spmd

    nc = _get_nc()
    in_maps = _prepare(eeg, fnirs, Wq, bq, Wk, bk, Wv, bv)
    res = run_bass_kernel_spmd(nc, in_maps, list(range(N_CORES)), **spmd_kwargs)
    return _unpack_output([res.results[c]["OUT"] for c in range(N_CORES)]), res


def kernel(eeg, fnirs, Wq, bq, Wk, bk, Wv, bv):
    return _run(eeg, fnirs, Wq, bq, Wk, bk, Wv, bv)[0]


# revision 24
# speedup vs baseline: 1.0310x; 1.0214x over previous
"""CrossAttentionFusion Bass kernel v2 — feature-major end-to-end.

Reference (T=4096, B=64, D=64):
    q = eeg @ Wq.T + bq ; k = fnirs @ Wk.T + bk ; v = fnirs @ Wv.T + bv
    s = sum(q*k, -1) * D**-0.5 ; a = softmax(s, axis=0) ; out = eeg + a*v

Algebra: s = SCALE*(x^T G y + u.y + w.x), G = Wq^T Wk, u = Wk^T bq,
w = Wq^T bk (constant bq.bk dropped - softmax invariant).

Per core: 8 batches as 4 pairs; feature-major pair-packed tiles
[128 = 64f x 2 batches, tokens]. Token-chunks of 1024 (4 per pair).

Phase A (per pair p, block tb):
  z = BD(G)^T @ x        (PE -> PSUM)
  zs = z + u2            (ACT Identity, bias=u2, bf16 out)
  m = zs * y             (DVE tt 2x bf16)
  s[8rows@32*tb] += red8_p^T @ m + wsel_p^T @ x   (PE, rows 2p,2p+1)
  after p==3: E = exp(s) (ACT on 8-row slices, accum Z-partials)
Z: 4 selector matmuls -> Zrep per pair layout; DVE reciprocal -> rz2_p.
Phase B (per p, tb):
  Eb = onesel_p^T @ E-rows   (PE broadcast of unnormalized exp)
  ya = Eb * y                (DVE/ACT-route/GPS-route)
  v' = BD(Wv^T)^T @ ya + bvsel_p^T @ E-rows  (PE)
  o = (v' * rz2_p) + x       (DVE stt; rz per-partition batch-half)
  DMA out (feature-major bf16; host unpacks).
"""

import sys

sys.path.insert(0, "/opt/trn_rl_repo")

import ml_dtypes
import numpy as np

import concourse.bass as bass
import concourse.tile as tile
from concourse import bacc, mybir

T, B, D = 4096, 64, 64
N_CORES = 8
BC = B // N_CORES  # 8 batches per core
NP = BC // 2  # 4 pairs
NTB = 4  # token blocks of 1024
CH = T // NTB  # 1024
SCALE = float(D) ** -0.5
F32 = mybir.dt.float32
BF16 = mybir.dt.bfloat16
NPBF16 = ml_dtypes.bfloat16
AF = mybir.ActivationFunctionType
ALU = mybir.AluOpType

# routing: which (p, tb) chunk indices (0..15) take the ACT-evac path
YA_ACT = {1, 3, 5, 7, 9, 11, 13, 15}
O_ACT = {0, 2, 4, 6, 8, 10, 12, 14}

_CACHE = {}


def _build_nc():
    nc = bacc.Bacc(
        "TRN2", target_bir_lowering=False, debug=False, num_devices=N_CORES
    )

    xp_d = nc.dram_tensor("XP", [NP, 128, NTB * CH], BF16, kind="ExternalInput").ap()
    yp_d = nc.dram_tensor("YP", [NP, 128, NTB * CH], BF16, kind="ExternalInput").ap()
    bdg_d = nc.dram_tensor("BDG", [128, 128], BF16, kind="ExternalInput").ap()
    u2_d = nc.dram_tensor("U2", [128, 1], F32, kind="ExternalInput").ap()
    redw_d = nc.dram_tensor("REDW", [128, NP * 16], BF16, kind="ExternalInput").ap()
    onesel_d = nc.dram_tensor("ONESEL", [128, NP * 128], BF16, kind="ExternalInput").ap()
    zsel_d = nc.dram_tensor("ZSEL", [128, NP * 128], F32, kind="ExternalInput").ap()
    bdwv_d = nc.dram_tensor("BDWV", [128, 128], BF16, kind="ExternalInput").ap()
    out_d = nc.dram_tensor("OUT", [NP, 128, NTB * CH], BF16, kind="ExternalOutput").ap()

    with tile.TileContext(nc) as tc:
        with (
            tc.tile_pool(name="consts", bufs=1) as consts,
            tc.tile_pool(name="store", bufs=1) as store,
            tc.tile_pool(name="zs", bufs=3) as zsp,
            tc.tile_pool(name="m", bufs=3) as mp,
            tc.tile_pool(name="eb", bufs=4) as ebp,
            tc.tile_pool(name="ya", bufs=16) as yap,
            tc.tile_pool(name="vs", bufs=3) as vsp,
            tc.tile_pool(name="o", bufs=3) as op_,
            tc.tile_pool(name="small", bufs=1) as smalls,
            # PSUM: pza 2x[128,1024]f32 = 4 banks (z in A, Eb in B)
            #       psv 2x[128,1024]f32 = 4 banks (s + Zrep in A, v in B)
            tc.tile_pool(name="pza", bufs=2, space="PSUM") as pza,
            tc.tile_pool(name="psv", bufs=2, space="PSUM") as psv,
        ):
            bdg_s = consts.tile([128, 128], BF16)
            nc.scalar.dma_start(bdg_s[:], bdg_d[:])
            u2_s = consts.tile([128, 1], F32)
            nc.scalar.dma_start(u2_s[:], u2_d[:])
            redw_s = consts.tile([128, NP * 16], BF16)
            nc.scalar.dma_start(redw_s[:], redw_d[:])
            onesel_s = consts.tile([128, NP * 128], BF16)
            nc.scalar.dma_start(onesel_s[:], onesel_d[:])
            zsel_s = consts.tile([128, NP * 128], F32)
            nc.scalar.dma_start(zsel_s[:], zsel_d[:])
            bdwv_s = consts.tile([128, 128], BF16)
            nc.scalar.dma_start(bdwv_s[:], bdwv_d[:])

            xstore = store.tile([128, NP * NTB * CH], BF16)  # 32 KB/part
            ystore = store.tile([128, NP * NTB * CH], BF16)  # 32 KB/part
            ostore = store.tile([128, NP * NTB * CH], BF16)  # 32 KB/part
            # E rows: tile tb//2, partition base 64*(tb%2), rows +2p+bi
            e_s = [store.tile([128, CH], BF16, name=f"e{h}") for h in range(2)]
            zp_s = smalls.tile([128, NTB], F32)  # Z partials per tb col
            rz2 = smalls.tile([128, NP], F32)  # 1/Z per pair, partition-half

            def csl(p, tb):
                base = (p * NTB + tb) * CH
                return slice(base, base + CH)

            nc.vector.memset(zp_s[:], 0.0)

            def mm512(out, lhsT, rhs, start, stop, **kw):
                # matmul out free dim must fit one PSUM bank (512 f32)
                n = out.shape[-1]
                for c0 in range(0, n, 512):
                    c1 = min(c0 + 512, n)
                    nc.tensor.matmul(
                        out[:, c0:c1], lhsT, rhs[:, c0:c1],
                        start=start, stop=stop, **kw,
                    )

            # s psum: tile tb//2, rows (2p+bi)@64*(tb%2); cols = token in block
            s_ps = [psv.tile([128, CH], F32, tag="sv", name=f"s{h}") for h in range(2)]

            # input DMAs: x on sync ring, y on gpsimd ring. Keep
            # descriptors >= 4 KB/partition; first pair in halves so the
            # first block lands early
            for h in range(2):
                cs = slice(h * 2 * CH, (h + 1) * 2 * CH)
                nc.sync.dma_start(xstore[:, cs], xp_d[0, :, cs])
                nc.gpsimd.dma_start(ystore[:, cs], yp_d[0, :, cs])
            for p in range(1, NP):
                psl = slice(p * NTB * CH, (p + 1) * NTB * CH)
                nc.sync.dma_start(xstore[:, psl], xp_d[p])
                nc.gpsimd.dma_start(ystore[:, psl], yp_d[p])

            # ---------------- Phase A (lag-2 software pipeline on PE) -----
            LAG = 2
            m_tiles = {}

            def a_front(i):
                p, tb = divmod(i, NTB)
                sl = csl(p, tb)
                zq = pza.tile([128, CH], F32, tag="za", name="zq")
                mm512(zq, bdg_s[:], xstore[:, sl], start=True, stop=True)
                zs = zsp.tile([128, CH], BF16, tag="zs")
                nc.scalar.activation(
                    zs[:], zq[:], AF.Identity, bias=u2_s[:, 0:1], scale=1.0
                )
                m = mp.tile([128, CH], BF16, tag="m")
                nc.vector.tensor_tensor(m[:], zs[:], ystore[:, sl], op=ALU.mult)
                m_tiles[i] = m

            def a_back(i):
                p, tb = divmod(i, NTB)
                sl = csl(p, tb)
                m = m_tiles.pop(i)
                sh, prow = s_ps[tb // 2], slice(64 * (tb % 2), 64 * (tb % 2) + 8)
                mm512(
                    sh[prow, :], redw_s[:, p * 16 : p * 16 + 8], m[:],
                    start=(p == 0), stop=False, skip_group_check=True,
                )
                mm512(
                    sh[prow, :], redw_s[:, p * 16 + 8 : p * 16 + 16],
                    xstore[:, sl],
                    start=False, stop=(p == 3), skip_group_check=True,
                )
                if p == 3:
                    nc.scalar.activation(
                        e_s[tb // 2][prow, :], sh[prow, :], AF.Exp,
                        bias=0.0, scale=1.0,
                        accum_out=zp_s[prow, tb : tb + 1],
                    )

            for i in range(NP * NTB + LAG):
                if i < NP * NTB:
                    a_front(i)
                if i >= LAG:
                    a_back(i - LAG)

            # Z -> rz2 block, emitted inside phase B (see emit_z below) so
            # eb matmuls can start on PE before the Z reduction completes
            def emit_z():
                zp1 = smalls.tile([128, 1], F32)
                nc.vector.tensor_reduce(
                    zp1[:], zp_s[:], axis=mybir.AxisListType.X, op=ALU.add
                )
                zrep = pza.tile([128, CH], F32, tag="za", name="zrep")
                for p in range(NP):
                    nc.tensor.matmul(
                        zrep[:, p : p + 1],
                        zsel_s[:, p * 128 : (p + 1) * 128],
                        zp1[:, 0:1],
                        start=True, stop=True, skip_group_check=True,
                    )
                nc.vector.reciprocal(rz2[:, 0:NP], zrep[:, 0:NP])

            # ---------------- Phase B ------------------------------------
            # b_front(tb, p): Eb broadcast via SBUF->SBUF DMA (two E rows
            # replicated across the 64-partition halves), then ya = Eb * y
            # (DVE 2x). Emitted eagerly right after exp(tb) so it overlaps
            # the tail of phase A. b_back needs rz (full-T softmax sum).
            ya_tiles = {}

            def b_front(tb, p):
                ci = p * NTB + tb
                sl = csl(p, tb)
                et = e_s[tb // 2]
                prow = slice(64 * (tb % 2), 64 * (tb % 2) + 8)
                eb = pza.tile([128, CH], F32, tag="za", name="eb")
                mm512(
                    eb, onesel_s[prow, p * 128 : (p + 1) * 128],
                    et[prow, :], start=True, stop=True,
                )
                ya = yap.tile([128, CH], BF16, tag="ya")
                if ci in YA_ACT:
                    ebs = ebp.tile([128, CH], BF16, tag="ebs")
                    nc.scalar.activation(
                        ebs[:], eb[:], AF.Copy, bias=0.0, scale=1.0
                    )
                    nc.vector.tensor_tensor(
                        ya[:], ebs[:], ystore[:, sl], op=ALU.mult
                    )
                else:
                    nc.vector.tensor_tensor(
                        ya[:], eb[:], ystore[:, sl], op=ALU.mult
                    )
                ya_tiles[(tb, p)] = ya

            def b_back(tb, p):
                ci = p * NTB + tb
                sl = csl(p, tb)
                ya = ya_tiles.pop((tb, p))
                vps = psv.tile([128, CH], F32, tag="sv", name="vps")
                mm512(vps, bdwv_s[:], ya[:], start=True, stop=True)
                if ci in O_ACT:
                    vsn = vsp.tile([128, CH], BF16, tag="vsn")
                    nc.scalar.activation(
                        vsn[:], vps[:], AF.Copy, bias=0.0,
                        scale=rz2[:, p : p + 1],
                    )
                    nc.vector.tensor_tensor(
                        ostore[:, sl], vsn[:], xstore[:, sl], op=ALU.add
                    )
                else:
                    nc.vector.scalar_tensor_tensor(
                        ostore[:, sl], vps[:], rz2[:, p : p + 1],
                        xstore[:, sl],
                        op0=ALU.mult, op1=ALU.add,
                    )
                if tb % 2 == 1:
                    h = tb // 2
                    hs = slice((2 * h) * CH, (2 * h + 2) * CH)
                    psl = slice((p * NTB + 2 * h) * CH, (p * NTB + 2 * h + 2) * CH)
                    nc.gpsimd.dma_start(out_d[p, :, hs], ostore[:, psl])

            emit_z()
            order = [(tb, p) for tb in range(NTB) for p in range(NP)]
            BLAG = 4
            for k in range(len(order) + BLAG):
                if k < len(order):
                    b_front(*order[k])
                if k >= BLAG:
                    b_back(*order[k - BLAG])

    nc.compile()
    return nc


def _get_nc():
    if "nc" not in _CACHE:
        _CACHE["nc"] = _build_nc()
    return _CACHE["nc"]


def _host_constants(Wq, bq, Wk, bk, Wv, bv):
    Wq64, Wk64, Wv64 = (np.asarray(a, np.float64) for a in (Wq, Wk, Wv))
    bq64, bk64, bv64 = (np.asarray(a, np.float64) for a in (bq, bk, bv))
    # bv fold: y'' = y + cv with cv = Wv^-1 bv, so v = Wv y'' exactly;
    # scores compensate via w' = w + G cv (u.cv constant drops in softmax)
    cv = np.linalg.solve(Wv64, bv64)
    G = SCALE * (Wq64.T @ Wk64)
    u = SCALE * (Wk64.T @ bq64)
    w = SCALE * (Wq64.T @ bk64) - G @ cv

    BDG = np.zeros((128, 128), np.float64)
    BDG[0:64, 0:64] = G
    BDG[64:128, 64:128] = G
    U2 = np.concatenate([u, u]).reshape(128, 1).astype(np.float32)
    # REDW: per pair p, cols [16p:16p+8] = m-reduce selector (col r = 2p+bi
    # -> ones at partitions bi*64..), cols [16p+8:16p+16] = w-vector selector
    REDW = np.zeros((128, NP * 16), np.float64)
    for p in range(NP):
        for bi in range(2):
            r = 2 * p + bi
            REDW[bi * 64 : (bi + 1) * 64, 16 * p + r] = 1.0
            REDW[bi * 64 : (bi + 1) * 64, 16 * p + 8 + r] = w
    # ONESEL: [128, 128] per pair, replicated at 2 64-row groups:
    # lhsT[64g + r, bi*64 + f] = 1 iff r == 2p+bi
    ONESEL = np.zeros((128, NP * 128), np.float64)
    for p in range(NP):
        for g in range(2):
            for bi in range(2):
                r = 64 * g + 2 * p + bi
                fs = slice(p * 128 + bi * 64, p * 128 + (bi + 1) * 64)
                ONESEL[r, fs] = 1.0
    # ZSEL: per pair p: zsel_p[64g + 2p + bi, bi*64 + f] = 1 (both groups;
    # zp rows outside the written group are zeroed by memset)
    ZSEL = np.zeros((128, NP * 128), np.float64)
    for p in range(NP):
        for g in range(2):
            for bi in range(2):
                ZSEL[64 * g + 2 * p + bi, p * 128 + bi * 64 : p * 128 + (bi + 1) * 64] = 1.0
    BDWV = np.zeros((128, 128), np.float64)
    BDWV[0:64, 0:64] = Wv64.T
    BDWV[64:128, 64:128] = Wv64.T
    return (
        BDG.astype(NPBF16), U2, REDW.astype(NPBF16),
        ONESEL.astype(NPBF16),
        ZSEL.astype(np.float32), BDWV.astype(NPBF16), cv,
    )


def _pack_inputs(eeg, fnirs, cv):
    # [core, p, tb, 128 = bi*64 + d, j]; token t = tb*1024 + j
    e = np.asarray(eeg, np.float32).reshape(NTB, CH, N_CORES, NP, 2, D)
    f = (np.asarray(fnirs, np.float64) + cv).astype(np.float32)
    f = f.reshape(NTB, CH, N_CORES, NP, 2, D)
    XP = np.ascontiguousarray(e.transpose(2, 3, 0, 4, 5, 1)).astype(NPBF16)
    YP = np.ascontiguousarray(f.transpose(2, 3, 0, 4, 5, 1)).astype(NPBF16)
    # [core, p, tb, 128, j] -> [core, p, 128, tb*CH]
    XP = XP.reshape(N_CORES, NP, NTB, 128, CH).transpose(0, 1, 3, 2, 4)
    YP = YP.reshape(N_CORES, NP, NTB, 128, CH).transpose(0, 1, 3, 2, 4)
    return (
        np.ascontiguousarray(XP).reshape(N_CORES, NP, 128, NTB * CH),
        np.ascontiguousarray(YP).reshape(N_CORES, NP, 128, NTB * CH),
    )


def _unpack_output(outs):
    o = np.stack(outs).astype(np.float32)  # [core, p, 128, tb*CH]
    o = o.reshape(N_CORES, NP, 2, D, NTB, CH)
    o = o.transpose(4, 5, 0, 1, 2, 3)  # [tb, j, core, p, bi, d]
    return np.ascontiguousarray(o.reshape(T, B, D))


def _prepare(eeg, fnirs, Wq, bq, Wk, bk, Wv, bv):
    BDG, U2, REDW, ONESEL, ZSEL, BDWV, cv = _host_constants(
        Wq, bq, Wk, bk, Wv, bv
    )
    XP, YP = _pack_inputs(eeg, fnirs, cv)
    return [
        {
            "XP": XP[c], "YP": YP[c], "BDG": BDG, "U2": U2, "REDW": REDW,
            "ONESEL": ONESEL, "ZSEL": ZSEL, "BDWV": BDWV,
        }
        for c in range(N_CORES)
    ]


def _run(eeg, fnirs, Wq, bq, Wk, bk, Wv, bv, **spmd_kwargs):
    from concourse.bass_utils import run_bass_kernel_spmd

    nc = _get_nc()
    in_maps = _prepare(eeg, fnirs, Wq, bq, Wk, bk, Wv, bv)
    res = run_bass_kernel_spmd(nc, in_maps, list(range(N_CORES)), **spmd_kwargs)
    return _unpack_output([res.results[c]["OUT"] for c in range(N_CORES)]), res


def kernel(eeg, fnirs, Wq, bq, Wk, bk, Wv, bv):
    return _run(eeg, fnirs, Wq, bq, Wk, bk, Wv, bv)[0]
